# revision 23
# baseline (speedup 1.0000x reference)
"""Per-sample modulated Conv3D (B=4, CIN=COUT=16, T=16, H=W=128, K=3x3x3)
on 8 TRN2 NeuronCores.

Sharding: data-parallel over (batch, T-half) -> 8 shards, no cross-core
communication. Each core computes a [16, 8, 128, 128] output slab.

Mapping: banded im2col in bf16. One matmul column = a base output
location (hb, w) with h = 4*hb + dh; the M dim packs
(dt in 2, dh in 4, co in 16) = 128 outputs per column; the K dim packs
(plane-group g in 4, kh~-pair in 2, ci in 16) = 128 rows across 3
chunk positions (kh~ in 6 total); kw in 3 via free-dim offsets into
w-padded rows. 9 accumulated bf16 matmuls of [K=128, M=128, N=512] per
PSUM tile (or 18 of K=64 for the slot-straddling windows).

All 10 input planes stay resident in SBUF as 9 bf16 tiles
Q[slot][c] (slot = tau//4, partition g = tau%4), so each plane's
kh~-replicated rows are DMA'd exactly once. Windows with tb%4==0 read
one slot uniformly; tb%4==2 windows read two slots as partition
halves, which auto-derive PE row-groups 0/64 and run concurrently; a
kt~-rotated weight copy keeps lhsT/rhs partition bases matched.

The host pre-replicates rows (x_rep[tau, kh~, ci, hb, w'] =
x[ci, tau, 4*hb + kh~, w']) and converts to bf16; the output leaves in
the staging layout (split over both HWDGE rings) and the host undoes
the permutation.
"""

import math

import numpy as np
import ml_dtypes

import concourse.bacc as bacc
import concourse.mybir as mybir
from concourse.tile import TileContext
from concourse.bass_utils import run_bass_kernel_spmd

B, CIN, COUT = 4, 16, 16
T, H, W = 16, 128, 128
K = 3
SCALE = 1.0 / math.sqrt(CIN * K * K * K)

N_CORES = 8
TSH = T // 2          # output t-planes per core
TAU = TSH + 2         # input planes per core (with halo)
HP = H + 2            # padded h'
WP = W + 2            # padded w'
DT, DH = 2, 4         # M-dim banding
NB = H // DH          # hb grid (32)
NT = NB // 4          # n-tiles per t_b (8, 4 hb each)
FREE = NB * WP        # per-partition free size of a chunk tile (4160)

_cache = {}


def _build_program():
    f32 = mybir.dt.float32
    bf16 = mybir.dt.bfloat16
    nc = bacc.Bacc("TRN2", target_bir_lowering=False, debug=False)
    # x: per-tile flat layout [slot, c, p=(g,khl,ci), (hb,w')] with
    # row h' = 4*hb + (2c+khl), plane tau = 4*slot + g (bf16).
    x = nc.dram_tensor("x", [3, 3, 128, FREE], bf16, kind="ExternalInput")
    # wb[0] = banded weights (K-row = kt~*32 + khl*16 + ci); wb[1] = same
    # rolled by 64 K-rows (kt~ -> kt~-2) for the slot-straddling windows.
    wb = nc.dram_tensor("wb", [2, 128, 9 * 128], bf16, kind="ExternalInput")
    # o: staging-layout output [tb, half, quarter, p64, (jl hbl w)],
    # each (tb, half, quarter) block contiguous in DRAM.
    o = nc.dram_tensor(
        "o", [TSH // DT, 2, 4, 64, 1024], f32, kind="ExternalOutput"
    )

    with TileContext(nc) as tc:
        with (
            tc.tile_pool(name="wt", bufs=1) as wt_pool,
            tc.tile_pool(name="q", bufs=1) as q_pool,
            tc.tile_pool(name="ps", bufs=4, space="PSUM") as ps_pool,
            tc.tile_pool(name="st", bufs=2) as st_pool,
        ):
            wt = wt_pool.tile([128, 9 * 128], bf16, name="wt")
            wtr = wt_pool.tile([128, 9 * 128], bf16, name="wtr")
            nc.sync.dma_start(out=wt[:], in_=wb[0])
            nc.scalar.dma_start(out=wtr[:], in_=wb[1])

            # Resident input tiles Q[slot][c]; partition p = g*32+khl*16+ci,
            # holding plane tau = 4*slot + g, row h' = 4*hb + (2c+khl).
            q = [[q_pool.tile([128, FREE], bf16, name=f"q{s}{c}")
                  for c in range(3)] for s in range(3)]
            eng = [nc.sync, nc.scalar]
            # slot-0 tiles first (they gate the first window), then the rest;
            # one flat-source DMA per tile (fast DMA path); slot 2 only has
            # planes 8,9 -> lower half
            for n, (s, c) in enumerate(
                [(0, 0), (0, 1), (0, 2), (1, 0), (1, 1), (1, 2),
                 (2, 0), (2, 1), (2, 2)]
            ):
                e = eng[n % 2]
                if s < 2:
                    e.dma_start(out=q[s][c][:], in_=x[s, c])
                else:
                    e.dma_start(out=q[s][c][0:64], in_=x[s, c, 0:64])

            for wi, tb in enumerate((0, 2, 6, 4)):
                tbi = tb // DT
                final = wi == 3
                uniform = tb % 4 == 0
                slot = tb // 4
                st = st_pool.tile([128, NT * 512], f32, name="st")
                for j in range(NT):
                    stj = st[:, j * 512 : (j + 1) * 512]
                    if uniform:
                        ps = ps_pool.tile([128, 512], f32, name="ps", tag="psA", bufs=3)
                        for s in range(9):
                            c, kw = divmod(s, 3)
                            rhs = q[slot][c].rearrange(
                                "p (hb w) -> p hb w", w=WP
                            )[:, 4 * j : 4 * j + 4, kw : kw + W]
                            nc.tensor.matmul(
                                ps[:],
                                lhsT=wt[:, s * 128 : (s + 1) * 128],
                                rhs=rhs,
                                start=(s == 0),
                                stop=(s == 8),
                            )
                        nc.vector.tensor_copy(out=stj, in_=ps[:])
                    else:
                        # Window straddles two slots. Lower partition half
                        # (g 0,1) = planes of the next slot (kt~ 2,3);
                        # upper half (g 2,3) = current slot (kt~ 0,1).
                        # One PSUM accumulation group per PE row-group
                        # (multiple matmuls per row-group in a single
                        # group fail on HW); the pairs run concurrently.
                        psA = ps_pool.tile([128, 512], f32, name="psA", tag="psA", bufs=3)
                        psB = ps_pool.tile([128, 512], f32, name="psB", tag="psB", bufs=3)
                        for s in range(9):
                            c, kw = divmod(s, 3)
                            s128 = s * 128
                            rA = q[slot + 1][c].rearrange(
                                "p (hb w) -> p hb w", w=WP
                            )[0:64, 4 * j : 4 * j + 4, kw : kw + W]
                            nc.tensor.matmul(
                                psA[:],
                                lhsT=wtr[0:64, s128 : s128 + 128],
                                rhs=rA,
                                start=(s == 0),
                                stop=(s == 8),
                            )
                            rB = q[slot][c].rearrange(
                                "p (hb w) -> p hb w", w=WP
                            )[64:128, 4 * j : 4 * j + 4, kw : kw + W]
                            nc.tensor.matmul(
                                psB[:],
                                lhsT=wtr[64:128, s128 : s128 + 128],
                                rhs=rB,
                                start=(s == 0),
                                stop=(s == 8),
                            )
                        nc.vector.tensor_copy(out=stj, in_=psA[:])
                        nc.vector.tensor_add(out=stj, in0=stj, in1=psB[:])
                    if final:
                        # evict per-j so the last write + HBM receipt is small
                        qt, jl = divmod(j, 2)
                        f0, f1 = j * 512, (j + 1) * 512
                        g0, g1 = jl * 512, (jl + 1) * 512
                        nc.sync.dma_start(
                            out=o[tbi, 0, qt, :, g0:g1], in_=st[0:64, f0:f1]
                        )
                        nc.scalar.dma_start(
                            out=o[tbi, 1, qt, :, g0:g1], in_=st[64:128, f0:f1]
                        )
                    elif j % 2 == 1:
                        # evict the finished quarter so the final DMA (and
                        # its HBM write receipt) isn't one big tail event
                        qt = j // 2
                        f0, f1 = (j - 1) * 512, (j + 1) * 512
                        nc.sync.dma_start(
                            out=o[tbi, 0, qt], in_=st[0:64, f0:f1]
                        )
                        nc.scalar.dma_start(
                            out=o[tbi, 1, qt], in_=st[64:128, f0:f1]
                        )
    nc.compile()
    return nc


def _prep_inputs(input, condition_feature, weight):
    """Host-side shard + layout prep. Returns in_maps for the 8 cores."""
    x_pad = np.zeros((B, CIN, T + 2, HP, WP), np.float32)
    x_pad[:, :, 1 : T + 1, 1 : H + 1, 1 : W + 1] = input

    # Per-sample modulated weights: [B, CO, CI, K, K, K]
    wmod = (weight[None] * SCALE * condition_feature).astype(np.float32)

    in_maps = []
    for core in range(N_CORES):
        b, half = divmod(core, 2)
        sh = x_pad[b, :, 8 * half : 8 * half + TAU].transpose(1, 0, 2, 3)
        x_rep = np.empty((TAU, 6, CIN, NB, WP), np.float32)
        for kh in range(6):
            x_rep[:, kh] = sh[:, :, kh : kh + 4 * NB : 4, :]
        x_t = np.zeros((3, 3, 4, 2, CIN, FREE), np.float32)
        for tau in range(TAU):
            s, g = divmod(tau, 4)
            x_t[s, :, g] = x_rep[tau].reshape(3, 2, CIN, FREE)
        wm = wmod[b]
        Wb = np.zeros((128, 3, 3, 128), np.float32)  # [K, c, kw, M]
        for c in range(3):
            for kw in range(3):
                for kt_t in range(4):
                    for khl in range(2):
                        kh_t = 2 * c + khl
                        for dt in range(DT):
                            kt = kt_t - dt
                            if not 0 <= kt < K:
                                continue
                            for dh in range(DH):
                                kh = kh_t - dh
                                if not 0 <= kh < K:
                                    continue
                                K0 = kt_t * 32 + khl * 16
                                M0 = dt * 64 + dh * 16
                                Wb[K0 : K0 + 16, c, kw, M0 : M0 + 16] = wm[
                                    :, :, kt, kh, kw
                                ].T
        Wb = Wb.reshape(128, 9 * 128)
        wb2 = np.stack([Wb, np.roll(Wb, -64, axis=0)])
        in_maps.append(
            {
                "x": np.ascontiguousarray(
                    x_t.reshape(3, 3, 128, FREE).astype(ml_dtypes.bfloat16)
                ),
                "wb": np.ascontiguousarray(wb2.astype(ml_dtypes.bfloat16)),
            }
        )
    return in_maps


def _run(inputs, trace=False, tmpdir=None, trace_cores=None):
    if "nc" not in _cache:
        _cache["nc"] = _build_program()
    nc = _cache["nc"]
    in_maps = _prep_inputs(**inputs)
    res = run_bass_kernel_spmd(
        nc,
        in_maps,
        core_ids=list(range(N_CORES)),
        trace=trace,
        tmpdir=tmpdir,
        trace_cores=trace_cores,
    )
    out = np.empty((B, COUT, T, H, W), np.float32)
    for core in range(N_CORES):
        b, half = divmod(core, 2)
        # o: [tbi, dt, qt, (dh co), (jl hbl w)] -> [co, t, h, w]
        arr = res.results[core]["o"].reshape(
            TSH // DT, DT, 4, DH, COUT, 2, 4, W
        )
        arr = arr.transpose(4, 0, 1, 2, 5, 6, 3, 7).reshape(COUT, TSH, H, W)
        out[b, :, 8 * half : 8 * half + TSH] = arr
    return out, res


def kernel(**inputs) -> np.ndarray:
    out, _ = _run(inputs, trace=False)
    return out


# revision 24
# speedup vs baseline: 1.0116x; 1.0116x over previous
"""Per-sample modulated Conv3D (B=4, CIN=COUT=16, T=16, H=W=128, K=3x3x3)
on 8 TRN2 NeuronCores.

Sharding: data-parallel over (batch, T-half) -> 8 shards, no cross-core
communication. Each core computes a [16, 8, 128, 128] output slab.

Mapping: banded im2col in bf16. One matmul column = a base output
location (hb, w) with h = 4*hb + dh; the M dim packs
(dt in 2, dh in 4, co in 16) = 128 outputs per column; the K dim packs
(plane-group g in 4, kh~-pair in 2, ci in 16) = 128 rows across 3
chunk positions (kh~ in 6 total); kw in 3 via free-dim offsets into
w-padded rows. 9 accumulated bf16 matmuls of [K=128, M=128, N=512] per
PSUM tile (or 18 of K=64 for the slot-straddling windows).

All 10 input planes stay resident in SBUF as 9 bf16 tiles
Q[slot][c] (slot = tau//4, partition g = tau%4), so each plane's
kh~-replicated rows are DMA'd exactly once. Windows with tb%4==0 read
one slot uniformly; tb%4==2 windows read two slots as partition
halves, which auto-derive PE row-groups 0/64 and run concurrently; a
kt~-rotated weight copy keeps lhsT/rhs partition bases matched.

The host pre-replicates rows (x_rep[tau, kh~, ci, hb, w'] =
x[ci, tau, 4*hb + kh~, w']) and converts to bf16; the output leaves in
the staging layout (split over both HWDGE rings) and the host undoes
the permutation.
"""

import math

import numpy as np
import ml_dtypes

import concourse.bacc as bacc
import concourse.mybir as mybir
from concourse.tile import TileContext
from concourse.bass_utils import run_bass_kernel_spmd

B, CIN, COUT = 4, 16, 16
T, H, W = 16, 128, 128
K = 3
SCALE = 1.0 / math.sqrt(CIN * K * K * K)

N_CORES = 8
TSH = T // 2          # output t-planes per core
TAU = TSH + 2         # input planes per core (with halo)
HP = H + 2            # padded h'
WP = W + 2            # padded w'
DT, DH = 2, 4         # M-dim banding
NB = H // DH          # hb grid (32)
NT = NB // 4          # n-tiles per t_b (8, 4 hb each)
FREE = NB * WP        # per-partition free size of a chunk tile (4160)

_cache = {}


def _build_program():
    f32 = mybir.dt.float32
    bf16 = mybir.dt.bfloat16
    nc = bacc.Bacc("TRN2", target_bir_lowering=False, debug=False)
    # x: per-tile flat layout [slot, c, p=(g,khl,ci), (hb,w')] with
    # row h' = 4*hb + (2c+khl), plane tau = 4*slot + g (bf16).
    x = nc.dram_tensor("x", [3, 3, 128, FREE], bf16, kind="ExternalInput")
    # wb[0] = banded weights (K-row = kt~*32 + khl*16 + ci); wb[1] = same
    # rolled by 64 K-rows (kt~ -> kt~-2) for the slot-straddling windows.
    wb = nc.dram_tensor("wb", [2, 128, 9 * 128], bf16, kind="ExternalInput")
    # o: staging-layout output [tb, half, quarter, p64, (jl hbl w)],
    # each (tb, half, quarter) block contiguous in DRAM.
    o = nc.dram_tensor(
        "o", [TSH // DT, 2, 4, 64, 1024], f32, kind="ExternalOutput"
    )

    with TileContext(nc) as tc:
        with (
            tc.tile_pool(name="wt", bufs=1) as wt_pool,
            tc.tile_pool(name="q", bufs=1) as q_pool,
            tc.tile_pool(name="ps", bufs=4, space="PSUM") as ps_pool,
            tc.tile_pool(name="st", bufs=2) as st_pool,
        ):
            wt = wt_pool.tile([128, 9 * 128], bf16, name="wt")
            wtr = wt_pool.tile([128, 9 * 128], bf16, name="wtr")
            nc.sync.dma_start(out=wt[:], in_=wb[0])
            nc.scalar.dma_start(out=wtr[:], in_=wb[1])

            # Resident input tiles Q[slot][c]; partition p = g*32+khl*16+ci,
            # holding plane tau = 4*slot + g, row h' = 4*hb + (2c+khl).
            q = [[q_pool.tile([128, FREE], bf16, name=f"q{s}{c}")
                  for c in range(3)] for s in range(3)]
            eng = [nc.sync, nc.scalar]
            # slot-0 tiles first (they gate the first window), then the rest;
            # one flat-source DMA per tile (fast DMA path); slot 2 only has
            # planes 8,9 -> lower half
            for n, (s, c) in enumerate(
                [(0, 0), (0, 1), (0, 2), (1, 0), (1, 1), (1, 2),
                 (2, 0), (2, 1), (2, 2)]
            ):
                e = eng[n % 2]
                if s < 2:
                    e.dma_start(out=q[s][c][:], in_=x[s, c])
                else:
                    e.dma_start(out=q[s][c][0:64], in_=x[s, c, 0:64])

            # Warm the PE (HAM un-throttle needs ~3.4us of activity) while
            # the input DMAs stream; garbage values into a scratch bank.
            warm = ps_pool.tile([128, 128], f32, name="warm", tag="warm", bufs=1)
            for i in range(16):
                nc.tensor.matmul(
                    warm[:], lhsT=wt[:, 0:128], rhs=wt[:, 0:128],
                    start=True, stop=True,
                )

            for wi, tb in enumerate((0, 2, 6, 4)):
                tbi = tb // DT
                final = wi == 3
                uniform = tb % 4 == 0
                slot = tb // 4
                st = st_pool.tile([128, NT * 512], f32, name="st")
                for j in range(NT):
                    stj = st[:, j * 512 : (j + 1) * 512]
                    if uniform:
                        ps = ps_pool.tile([128, 512], f32, name="ps", tag="psA", bufs=3)
                        for s in range(9):
                            c, kw = divmod(s, 3)
                            rhs = q[slot][c].rearrange(
                                "p (hb w) -> p hb w", w=WP
                            )[:, 4 * j : 4 * j + 4, kw : kw + W]
                            nc.tensor.matmul(
                                ps[:],
                                lhsT=wt[:, s * 128 : (s + 1) * 128],
                                rhs=rhs,
                                start=(s == 0),
                                stop=(s == 8),
                            )
                        nc.vector.tensor_copy(out=stj, in_=ps[:])
                    else:
                        # Window straddles two slots. Lower partition half
                        # (g 0,1) = planes of the next slot (kt~ 2,3);
                        # upper half (g 2,3) = current slot (kt~ 0,1).
                        # One PSUM accumulation group per PE row-group
                        # (multiple matmuls per row-group in a single
                        # group fail on HW); the pairs run concurrently.
                        psA = ps_pool.tile([128, 512], f32, name="psA", tag="psA", bufs=3)
                        psB = ps_pool.tile([128, 512], f32, name="psB", tag="psB", bufs=3)
                        for s in range(9):
                            c, kw = divmod(s, 3)
                            s128 = s * 128
                            rA = q[slot + 1][c].rearrange(
                                "p (hb w) -> p hb w", w=WP
                            )[0:64, 4 * j : 4 * j + 4, kw : kw + W]
                            nc.tensor.matmul(
                                psA[:],
                                lhsT=wtr[0:64, s128 : s128 + 128],
                                rhs=rA,
                                start=(s == 0),
                                stop=(s == 8),
                            )
                            rB = q[slot][c].rearrange(
                                "p (hb w) -> p hb w", w=WP
                            )[64:128, 4 * j : 4 * j + 4, kw : kw + W]
                            nc.tensor.matmul(
                                psB[:],
                                lhsT=wtr[64:128, s128 : s128 + 128],
                                rhs=rB,
                                start=(s == 0),
                                stop=(s == 8),
                            )
                        nc.vector.tensor_copy(out=stj, in_=psA[:])
                        nc.vector.tensor_add(out=stj, in0=stj, in1=psB[:])
                    if final:
                        # evict per-j so the last write + HBM receipt is small
                        qt, jl = divmod(j, 2)
                        f0, f1 = j * 512, (j + 1) * 512
                        g0, g1 = jl * 512, (jl + 1) * 512
                        nc.sync.dma_start(
                            out=o[tbi, 0, qt, :, g0:g1], in_=st[0:64, f0:f1]
                        )
                        nc.scalar.dma_start(
                            out=o[tbi, 1, qt, :, g0:g1], in_=st[64:128, f0:f1]
                        )
                    elif j % 2 == 1:
                        # evict the finished quarter so the final DMA (and
                        # its HBM write receipt) isn't one big tail event
                        qt = j // 2
                        f0, f1 = (j - 1) * 512, (j + 1) * 512
                        nc.sync.dma_start(
                            out=o[tbi, 0, qt], in_=st[0:64, f0:f1]
                        )
                        nc.scalar.dma_start(
                            out=o[tbi, 1, qt], in_=st[64:128, f0:f1]
                        )
    nc.compile()
    return nc


def _prep_inputs(input, condition_feature, weight):
    """Host-side shard + layout prep. Returns in_maps for the 8 cores."""
    x_pad = np.zeros((B, CIN, T + 2, HP, WP), np.float32)
    x_pad[:, :, 1 : T + 1, 1 : H + 1, 1 : W + 1] = input

    # Per-sample modulated weights: [B, CO, CI, K, K, K]
    wmod = (weight[None] * SCALE * condition_feature).astype(np.float32)

    in_maps = []
    for core in range(N_CORES):
        b, half = divmod(core, 2)
        sh = x_pad[b, :, 8 * half : 8 * half + TAU].transpose(1, 0, 2, 3)
        x_rep = np.empty((TAU, 6, CIN, NB, WP), np.float32)
        for kh in range(6):
            x_rep[:, kh] = sh[:, :, kh : kh + 4 * NB : 4, :]
        x_t = np.zeros((3, 3, 4, 2, CIN, FREE), np.float32)
        for tau in range(TAU):
            s, g = divmod(tau, 4)
            x_t[s, :, g] = x_rep[tau].reshape(3, 2, CIN, FREE)
        wm = wmod[b]
        Wb = np.zeros((128, 3, 3, 128), np.float32)  # [K, c, kw, M]
        for c in range(3):
            for kw in range(3):
                for kt_t in range(4):
                    for khl in range(2):
                        kh_t = 2 * c + khl
                        for dt in range(DT):
                            kt = kt_t - dt
                            if not 0 <= kt < K:
                                continue
                            for dh in range(DH):
                                kh = kh_t - dh
                                if not 0 <= kh < K:
                                    continue
                                K0 = kt_t * 32 + khl * 16
                                M0 = dt * 64 + dh * 16
                                Wb[K0 : K0 + 16, c, kw, M0 : M0 + 16] = wm[
                                    :, :, kt, kh, kw
                                ].T
        Wb = Wb.reshape(128, 9 * 128)
        wb2 = np.stack([Wb, np.roll(Wb, -64, axis=0)])
        in_maps.append(
            {
                "x": np.ascontiguousarray(
                    x_t.reshape(3, 3, 128, FREE).astype(ml_dtypes.bfloat16)
                ),
                "wb": np.ascontiguousarray(wb2.astype(ml_dtypes.bfloat16)),
            }
        )
    return in_maps


def _run(inputs, trace=False, tmpdir=None, trace_cores=None):
    if "nc" not in _cache:
        _cache["nc"] = _build_program()
    nc = _cache["nc"]
    in_maps = _prep_inputs(**inputs)
    res = run_bass_kernel_spmd(
        nc,
        in_maps,
        core_ids=list(range(N_CORES)),
        trace=trace,
        tmpdir=tmpdir,
        trace_cores=trace_cores,
    )
    out = np.empty((B, COUT, T, H, W), np.float32)
    for core in range(N_CORES):
        b, half = divmod(core, 2)
        # o: [tbi, dt, qt, (dh co), (jl hbl w)] -> [co, t, h, w]
        arr = res.results[core]["o"].reshape(
            TSH // DT, DT, 4, DH, COUT, 2, 4, W
        )
        arr = arr.transpose(4, 0, 1, 2, 5, 6, 3, 7).reshape(COUT, TSH, H, W)
        out[b, :, 8 * half : 8 * half + TSH] = arr
    return out, res


def kernel(**inputs) -> np.ndarray:
    out, _ = _run(inputs, trace=False)
    return out


# revision 25
# speedup vs baseline: 1.0183x; 1.0066x over previous
"""Per-sample modulated Conv3D (B=4, CIN=COUT=16, T=16, H=W=128, K=3x3x3)
on 8 TRN2 NeuronCores.

Sharding: data-parallel over (batch, T-half) -> 8 shards, no cross-core
communication. Each core computes a [16, 8, 128, 128] output slab.

Mapping: banded im2col in bf16. One matmul column = a base output
location (hb, w) with h = 4*hb + dh; the M dim packs
(dt in 2, dh in 4, co in 16) = 128 outputs per column; the K dim packs
(plane-group g in 4, kh~-pair in 2, ci in 16) = 128 rows across 3
chunk positions (kh~ in 6 total); kw in 3 via free-dim offsets into
w-padded rows. 9 accumulated bf16 matmuls of [K=128, M=128, N=512] per
PSUM tile (or 18 of K=64 for the slot-straddling windows).

All 10 input planes stay resident in SBUF as 9 bf16 tiles
Q[slot][c] (slot = tau//4, partition g = tau%4), so each plane's
kh~-replicated rows are DMA'd exactly once. Windows with tb%4==0 read
one slot uniformly; tb%4==2 windows read two slots as partition
halves, which auto-derive PE row-groups 0/64 and run concurrently; a
kt~-rotated weight copy keeps lhsT/rhs partition bases matched.

The host pre-replicates rows (x_rep[tau, kh~, ci, hb, w'] =
x[ci, tau, 4*hb + kh~, w']) and converts to bf16; the output leaves in
the staging layout (split over both HWDGE rings) and the host undoes
the permutation.
"""

import math

import numpy as np
import ml_dtypes

import concourse.bacc as bacc
import concourse.mybir as mybir
from concourse.tile import TileContext
from concourse.bass_utils import run_bass_kernel_spmd

B, CIN, COUT = 4, 16, 16
T, H, W = 16, 128, 128
K = 3
SCALE = 1.0 / math.sqrt(CIN * K * K * K)

N_CORES = 8
TSH = T // 2          # output t-planes per core
TAU = TSH + 2         # input planes per core (with halo)
HP = H + 2            # padded h'
WP = W + 2            # padded w'
DT, DH = 2, 4         # M-dim banding
NB = H // DH          # hb grid (32)
NT = NB // 4          # n-tiles per t_b (8, 4 hb each)
FREE = NB * WP        # per-partition free size of a chunk tile (4160)

_cache = {}


def _build_program():
    f32 = mybir.dt.float32
    bf16 = mybir.dt.bfloat16
    nc = bacc.Bacc("TRN2", target_bir_lowering=False, debug=False)
    # x: per-tile flat layout [slot, c, p=(g,khl,ci), (hb,w')] with
    # row h' = 4*hb + (2c+khl), plane tau = 4*slot + g (bf16).
    x = nc.dram_tensor("x", [3, 3, 128, FREE], bf16, kind="ExternalInput")
    # wb[0] = banded weights (K-row = kt~*32 + khl*16 + ci); wb[1] = same
    # rolled by 64 K-rows (kt~ -> kt~-2) for the slot-straddling windows.
    wb = nc.dram_tensor("wb", [2, 128, 9 * 128], bf16, kind="ExternalInput")
    # o: staging-layout output [tb, half, quarter, p64, (jl hbl w)],
    # each (tb, half, quarter) block contiguous in DRAM.
    o = nc.dram_tensor(
        "o", [TSH // DT, 2, 4, 64, 1024], f32, kind="ExternalOutput"
    )

    with TileContext(nc) as tc:
        with (
            tc.tile_pool(name="wt", bufs=1) as wt_pool,
            tc.tile_pool(name="q", bufs=1) as q_pool,
            tc.tile_pool(name="ps", bufs=4, space="PSUM") as ps_pool,
            tc.tile_pool(name="st", bufs=2) as st_pool,
        ):
            wt = wt_pool.tile([128, 9 * 128], bf16, name="wt")
            wtr = wt_pool.tile([128, 9 * 128], bf16, name="wtr")
            nc.sync.dma_start(out=wt[:], in_=wb[0])
            nc.scalar.dma_start(out=wtr[:], in_=wb[1])

            # Resident input tiles Q[slot][c]; partition p = g*32+khl*16+ci,
            # holding plane tau = 4*slot + g, row h' = 4*hb + (2c+khl).
            q = [[q_pool.tile([128, FREE], bf16, name=f"q{s}{c}")
                  for c in range(3)] for s in range(3)]
            eng = [nc.sync, nc.scalar]
            for s in range(3):
                for c in range(3):
                    # one flat-source DMA per tile (fast DMA path); slot 2
                    # only has planes 8,9 -> lower half
                    e = eng[(s * 3 + c) % 2]
                    if s < 2:
                        e.dma_start(out=q[s][c][:], in_=x[s, c])
                    else:
                        e.dma_start(out=q[s][c][0:64], in_=x[s, c, 0:64])

            # Warm the PE (HAM un-throttle needs ~3.4us of activity) while
            # the input DMAs stream; garbage values into a scratch bank.
            warm = ps_pool.tile([128, 128], f32, name="warm", tag="warm", bufs=1)
            for i in range(16):
                nc.tensor.matmul(
                    warm[:], lhsT=wt[:, 0:128], rhs=wt[:, 0:128],
                    start=True, stop=True,
                )

            for tbi in range(TSH // DT):
                tb = DT * tbi
                uniform = tb % 4 == 0
                slot = tb // 4
                st = st_pool.tile([128, NT * 512], f32, name="st")
                for j in range(NT):
                    stj = st[:, j * 512 : (j + 1) * 512]
                    if uniform:
                        ps = ps_pool.tile([128, 512], f32, name="ps", tag="psA", bufs=3)
                        for s in range(9):
                            c, kw = divmod(s, 3)
                            rhs = q[slot][c].rearrange(
                                "p (hb w) -> p hb w", w=WP
                            )[:, 4 * j : 4 * j + 4, kw : kw + W]
                            nc.tensor.matmul(
                                ps[:],
                                lhsT=wt[:, s * 128 : (s + 1) * 128],
                                rhs=rhs,
                                start=(s == 0),
                                stop=(s == 8),
                            )
                        nc.vector.tensor_copy(out=stj, in_=ps[:])
                    else:
                        # Window straddles two slots. Lower partition half
                        # (g 0,1) = planes of the next slot (kt~ 2,3);
                        # upper half (g 2,3) = current slot (kt~ 0,1).
                        # One PSUM accumulation group per PE row-group
                        # (multiple matmuls per row-group in a single
                        # group fail on HW); the pairs run concurrently.
                        psA = ps_pool.tile([128, 512], f32, name="psA", tag="psA", bufs=3)
                        psB = ps_pool.tile([128, 512], f32, name="psB", tag="psB", bufs=3)
                        for s in range(9):
                            c, kw = divmod(s, 3)
                            s128 = s * 128
                            rA = q[slot + 1][c].rearrange(
                                "p (hb w) -> p hb w", w=WP
                            )[0:64, 4 * j : 4 * j + 4, kw : kw + W]
                            nc.tensor.matmul(
                                psA[:],
                                lhsT=wtr[0:64, s128 : s128 + 128],
                                rhs=rA,
                                start=(s == 0),
                                stop=(s == 8),
                            )
                            rB = q[slot][c].rearrange(
                                "p (hb w) -> p hb w", w=WP
                            )[64:128, 4 * j : 4 * j + 4, kw : kw + W]
                            nc.tensor.matmul(
                                psB[:],
                                lhsT=wtr[64:128, s128 : s128 + 128],
                                rhs=rB,
                                start=(s == 0),
                                stop=(s == 8),
                            )
                        nc.vector.tensor_copy(out=stj, in_=psA[:])
                        nc.vector.tensor_add(out=stj, in0=stj, in1=psB[:])
                    if j % 2 == 1:
                        # evict the finished quarter so the final DMA (and
                        # its HBM write receipt) isn't one big tail event
                        qt = j // 2
                        f0, f1 = (j - 1) * 512, (j + 1) * 512
                        nc.sync.dma_start(
                            out=o[tbi, 0, qt], in_=st[0:64, f0:f1]
                        )
                        nc.scalar.dma_start(
                            out=o[tbi, 1, qt], in_=st[64:128, f0:f1]
                        )
    nc.compile()
    return nc


def _prep_inputs(input, condition_feature, weight):
    """Host-side shard + layout prep. Returns in_maps for the 8 cores."""
    x_pad = np.zeros((B, CIN, T + 2, HP, WP), np.float32)
    x_pad[:, :, 1 : T + 1, 1 : H + 1, 1 : W + 1] = input

    # Per-sample modulated weights: [B, CO, CI, K, K, K]
    wmod = (weight[None] * SCALE * condition_feature).astype(np.float32)

    in_maps = []
    for core in range(N_CORES):
        b, half = divmod(core, 2)
        sh = x_pad[b, :, 8 * half : 8 * half + TAU].transpose(1, 0, 2, 3)
        x_rep = np.empty((TAU, 6, CIN, NB, WP), np.float32)
        for kh in range(6):
            x_rep[:, kh] = sh[:, :, kh : kh + 4 * NB : 4, :]
        x_t = np.zeros((3, 3, 4, 2, CIN, FREE), np.float32)
        for tau in range(TAU):
            s, g = divmod(tau, 4)
            x_t[s, :, g] = x_rep[tau].reshape(3, 2, CIN, FREE)
        wm = wmod[b]
        Wb = np.zeros((128, 3, 3, 128), np.float32)  # [K, c, kw, M]
        for c in range(3):
            for kw in range(3):
                for kt_t in range(4):
                    for khl in range(2):
                        kh_t = 2 * c + khl
                        for dt in range(DT):
                            kt = kt_t - dt
                            if not 0 <= kt < K:
                                continue
                            for dh in range(DH):
                                kh = kh_t - dh
                                if not 0 <= kh < K:
                                    continue
                                K0 = kt_t * 32 + khl * 16
                                M0 = dt * 64 + dh * 16
                                Wb[K0 : K0 + 16, c, kw, M0 : M0 + 16] = wm[
                                    :, :, kt, kh, kw
                                ].T
        Wb = Wb.reshape(128, 9 * 128)
        wb2 = np.stack([Wb, np.roll(Wb, -64, axis=0)])
        in_maps.append(
            {
                "x": np.ascontiguousarray(
                    x_t.reshape(3, 3, 128, FREE).astype(ml_dtypes.bfloat16)
                ),
                "wb": np.ascontiguousarray(wb2.astype(ml_dtypes.bfloat16)),
            }
        )
    return in_maps


def _run(inputs, trace=False, tmpdir=None, trace_cores=None):
    if "nc" not in _cache:
        _cache["nc"] = _build_program()
    nc = _cache["nc"]
    in_maps = _prep_inputs(**inputs)
    res = run_bass_kernel_spmd(
        nc,
        in_maps,
        core_ids=list(range(N_CORES)),
        trace=trace,
        tmpdir=tmpdir,
        trace_cores=trace_cores,
    )
    out = np.empty((B, COUT, T, H, W), np.float32)
    for core in range(N_CORES):
        b, half = divmod(core, 2)
        # o: [tbi, dt, qt, (dh co), (jl hbl w)] -> [co, t, h, w]
        arr = res.results[core]["o"].reshape(
            TSH // DT, DT, 4, DH, COUT, 2, 4, W
        )
        arr = arr.transpose(4, 0, 1, 2, 5, 6, 3, 7).reshape(COUT, TSH, H, W)
        out[b, :, 8 * half : 8 * half + TSH] = arr
    return out, res


def kernel(**inputs) -> np.ndarray:
    out, _ = _run(inputs, trace=False)
    return out


# revision 26
# speedup vs baseline: 1.0461x; 1.0273x over previous
"""Per-sample modulated Conv3D (B=4, CIN=COUT=16, T=16, H=W=128, K=3x3x3)
on 8 TRN2 NeuronCores.

Sharding: data-parallel over (batch, T-half) -> 8 shards, no cross-core
communication. Each core computes a [16, 8, 128, 128] output slab.

Mapping: banded im2col in bf16. One matmul column = a base output
location (hb, w) with h = 4*hb + dh; the M dim packs
(dt in 2, dh in 4, co in 16) = 128 outputs per column; the K dim packs
(plane-group g in 4, kh~-pair in 2, ci in 16) = 128 rows across 3
chunk positions (kh~ in 6 total); kw in 3 via free-dim offsets into
w-padded rows. 9 accumulated bf16 matmuls of [K=128, M=128, N=512] per
PSUM tile (or 18 of K=64 for the slot-straddling windows).

All 10 input planes stay resident in SBUF as 9 bf16 tiles
Q[slot][c] (slot = tau//4, partition g = tau%4), so each plane's
kh~-replicated rows are DMA'd exactly once. Windows with tb%4==0 read
one slot uniformly; tb%4==2 windows read two slots as partition
halves, which auto-derive PE row-groups 0/64 and run concurrently; a
kt~-rotated weight copy keeps lhsT/rhs partition bases matched.

The host pre-replicates rows (x_rep[tau, kh~, ci, hb, w'] =
x[ci, tau, 4*hb + kh~, w']) and converts to bf16; the output leaves in
the staging layout (split over both HWDGE rings) and the host undoes
the permutation.
"""

import math

import numpy as np
import ml_dtypes

import concourse.bacc as bacc
import concourse.mybir as mybir
from concourse.tile import TileContext
from concourse.bass_utils import run_bass_kernel_spmd

B, CIN, COUT = 4, 16, 16
T, H, W = 16, 128, 128
K = 3
SCALE = 1.0 / math.sqrt(CIN * K * K * K)

N_CORES = 8
TSH = T // 2          # output t-planes per core
TAU = TSH + 2         # input planes per core (with halo)
HP = H + 2            # padded h'
WP = W + 2            # padded w'
DT, DH = 2, 4         # M-dim banding
NB = H // DH          # hb grid (32)
NT = NB // 4          # n-tiles per t_b (8, 4 hb each)
FREE = NB * WP        # per-partition free size of a chunk tile (4160)

_cache = {}


def _build_program():
    f32 = mybir.dt.float32
    bf16 = mybir.dt.bfloat16
    nc = bacc.Bacc("TRN2", target_bir_lowering=False, debug=False)
    # x: per-tile flat layout [slot, c, p=(g,khl,ci), (hb,w')] with
    # row h' = 4*hb + (2c+khl), plane tau = 4*slot + g (bf16).
    x = nc.dram_tensor("x", [3, 3, 128, FREE], bf16, kind="ExternalInput")
    # wb[0] = banded weights (K-row = kt~*32 + khl*16 + ci); wb[1] = same
    # rolled by 64 K-rows (kt~ -> kt~-2) for the slot-straddling windows.
    wb = nc.dram_tensor("wb", [2, 128, 9 * 128], bf16, kind="ExternalInput")
    # o: staging-layout output [tb, half, quarter, p64, (jl hbl w)],
    # each (tb, half, quarter) block contiguous in DRAM.
    o = nc.dram_tensor(
        "o", [TSH // DT, 2, 4, 64, 1024], bf16, kind="ExternalOutput"
    )

    with TileContext(nc) as tc:
        with (
            tc.tile_pool(name="wt", bufs=1) as wt_pool,
            tc.tile_pool(name="q", bufs=1) as q_pool,
            tc.tile_pool(name="ps", bufs=4, space="PSUM") as ps_pool,
            tc.tile_pool(name="st", bufs=2) as st_pool,
        ):
            wt = wt_pool.tile([128, 9 * 128], bf16, name="wt")
            wtr = wt_pool.tile([128, 9 * 128], bf16, name="wtr")
            nc.sync.dma_start(out=wt[:], in_=wb[0])
            nc.scalar.dma_start(out=wtr[:], in_=wb[1])

            # Resident input tiles Q[slot][c]; partition p = g*32+khl*16+ci,
            # holding plane tau = 4*slot + g, row h' = 4*hb + (2c+khl).
            q = [[q_pool.tile([128, FREE], bf16, name=f"q{s}{c}")
                  for c in range(3)] for s in range(3)]
            eng = [nc.sync, nc.scalar]
            for s in range(3):
                for c in range(3):
                    # one flat-source DMA per tile (fast DMA path); slot 2
                    # only has planes 8,9 -> lower half
                    e = eng[(s * 3 + c) % 2]
                    if s < 2:
                        e.dma_start(out=q[s][c][:], in_=x[s, c])
                    else:
                        e.dma_start(out=q[s][c][0:64], in_=x[s, c, 0:64])

            # Warm the PE (HAM un-throttle needs ~3.4us of activity) while
            # the input DMAs stream; garbage values into a scratch bank.
            warm = ps_pool.tile([128, 128], f32, name="warm", tag="warm", bufs=1)
            for i in range(16):
                nc.tensor.matmul(
                    warm[:], lhsT=wt[:, 0:128], rhs=wt[:, 0:128],
                    start=True, stop=True,
                )

            for tbi in range(TSH // DT):
                tb = DT * tbi
                uniform = tb % 4 == 0
                slot = tb // 4
                st = st_pool.tile([128, NT * 512], bf16, name="st")
                for j in range(NT):
                    stj = st[:, j * 512 : (j + 1) * 512]
                    if uniform:
                        ps = ps_pool.tile([128, 512], f32, name="ps", tag="psA", bufs=3)
                        for s in range(9):
                            c, kw = divmod(s, 3)
                            rhs = q[slot][c].rearrange(
                                "p (hb w) -> p hb w", w=WP
                            )[:, 4 * j : 4 * j + 4, kw : kw + W]
                            nc.tensor.matmul(
                                ps[:],
                                lhsT=wt[:, s * 128 : (s + 1) * 128],
                                rhs=rhs,
                                start=(s == 0),
                                stop=(s == 8),
                            )
                        nc.vector.tensor_copy(out=stj, in_=ps[:])
                    else:
                        # Window straddles two slots. Lower partition half
                        # (g 0,1) = planes of the next slot (kt~ 2,3);
                        # upper half (g 2,3) = current slot (kt~ 0,1).
                        # One PSUM accumulation group per PE row-group
                        # (multiple matmuls per row-group in a single
                        # group fail on HW); the pairs run concurrently.
                        psA = ps_pool.tile([128, 512], f32, name="psA", tag="psA", bufs=3)
                        psB = ps_pool.tile([128, 512], f32, name="psB", tag="psB", bufs=3)
                        for s in range(9):
                            c, kw = divmod(s, 3)
                            s128 = s * 128
                            rA = q[slot + 1][c].rearrange(
                                "p (hb w) -> p hb w", w=WP
                            )[0:64, 4 * j : 4 * j + 4, kw : kw + W]
                            nc.tensor.matmul(
                                psA[:],
                                lhsT=wtr[0:64, s128 : s128 + 128],
                                rhs=rA,
                                start=(s == 0),
                                stop=(s == 8),
                            )
                            rB = q[slot][c].rearrange(
                                "p (hb w) -> p hb w", w=WP
                            )[64:128, 4 * j : 4 * j + 4, kw : kw + W]
                            nc.tensor.matmul(
                                psB[:],
                                lhsT=wtr[64:128, s128 : s128 + 128],
                                rhs=rB,
                                start=(s == 0),
                                stop=(s == 8),
                            )
                        nc.vector.tensor_copy(out=stj, in_=psA[:])
                        nc.vector.tensor_add(out=stj, in0=stj, in1=psB[:])
                    if j % 2 == 1:
                        # evict the finished quarter so the final DMA (and
                        # its HBM write receipt) isn't one big tail event
                        qt = j // 2
                        f0, f1 = (j - 1) * 512, (j + 1) * 512
                        nc.sync.dma_start(
                            out=o[tbi, 0, qt], in_=st[0:64, f0:f1]
                        )
                        nc.scalar.dma_start(
                            out=o[tbi, 1, qt], in_=st[64:128, f0:f1]
                        )
    nc.compile()
    return nc


def _prep_inputs(input, condition_feature, weight):
    """Host-side shard + layout prep. Returns in_maps for the 8 cores."""
    x_pad = np.zeros((B, CIN, T + 2, HP, WP), np.float32)
    x_pad[:, :, 1 : T + 1, 1 : H + 1, 1 : W + 1] = input

    # Per-sample modulated weights: [B, CO, CI, K, K, K]
    wmod = (weight[None] * SCALE * condition_feature).astype(np.float32)

    in_maps = []
    for core in range(N_CORES):
        b, half = divmod(core, 2)
        sh = x_pad[b, :, 8 * half : 8 * half + TAU].transpose(1, 0, 2, 3)
        x_rep = np.empty((TAU, 6, CIN, NB, WP), np.float32)
        for kh in range(6):
            x_rep[:, kh] = sh[:, :, kh : kh + 4 * NB : 4, :]
        x_t = np.zeros((3, 3, 4, 2, CIN, FREE), np.float32)
        for tau in range(TAU):
            s, g = divmod(tau, 4)
            x_t[s, :, g] = x_rep[tau].reshape(3, 2, CIN, FREE)
        wm = wmod[b]
        Wb = np.zeros((128, 3, 3, 128), np.float32)  # [K, c, kw, M]
        for c in range(3):
            for kw in range(3):
                for kt_t in range(4):
                    for khl in range(2):
                        kh_t = 2 * c + khl
                        for dt in range(DT):
                            kt = kt_t - dt
                            if not 0 <= kt < K:
                                continue
                            for dh in range(DH):
                                kh = kh_t - dh
                                if not 0 <= kh < K:
                                    continue
                                K0 = kt_t * 32 + khl * 16
                                M0 = dt * 64 + dh * 16
                                Wb[K0 : K0 + 16, c, kw, M0 : M0 + 16] = wm[
                                    :, :, kt, kh, kw
                                ].T
        Wb = Wb.reshape(128, 9 * 128)
        wb2 = np.stack([Wb, np.roll(Wb, -64, axis=0)])
        in_maps.append(
            {
                "x": np.ascontiguousarray(
                    x_t.reshape(3, 3, 128, FREE).astype(ml_dtypes.bfloat16)
                ),
                "wb": np.ascontiguousarray(wb2.astype(ml_dtypes.bfloat16)),
            }
        )
    return in_maps


def _run(inputs, trace=False, tmpdir=None, trace_cores=None):
    if "nc" not in _cache:
        _cache["nc"] = _build_program()
    nc = _cache["nc"]
    in_maps = _prep_inputs(**inputs)
    res = run_bass_kernel_spmd(
        nc,
        in_maps,
        core_ids=list(range(N_CORES)),
        trace=trace,
        tmpdir=tmpdir,
        trace_cores=trace_cores,
    )
    out = np.empty((B, COUT, T, H, W), np.float32)
    for core in range(N_CORES):
        b, half = divmod(core, 2)
        # o: [tbi, dt, qt, (dh co), (jl hbl w)] -> [co, t, h, w]
        arr = res.results[core]["o"].astype(np.float32).reshape(
            TSH // DT, DT, 4, DH, COUT, 2, 4, W
        )
        arr = arr.transpose(4, 0, 1, 2, 5, 6, 3, 7).reshape(COUT, TSH, H, W)
        out[b, :, 8 * half : 8 * half + TSH] = arr
    return out, res


def kernel(**inputs) -> np.ndarray:
    out, _ = _run(inputs, trace=False)
    return out
